# revision 39
# baseline (speedup 1.0000x reference)
"""Trainium2 Bass kernel: DetectionCircle head + DIoU/focal loss.

Data-parallel over batch: 16 images per NeuronCore x 8 cores.
Per-core layout: partition p = b*8 + q  (b in 0..15, q in 0..7),
free dim covers cell = q*1352 + h*676 + f  (2 chunks of 676).

All box math is done in *grid units* (coords scaled by G=104) so that
the /G and *IMAGE_SIZE scales fold into constants and cell offsets
cancel in dx = sigmoid(x)_best - tx.  The circle-lens intersection uses
phi_i = atan2(sqrt(t), num_i) (the arccos of the reference) computed via
log-space range reduction so the ACT Arctan argument stays in [0,1]:
  r* = exp(-|ln|num| - 0.5 ln t|),  alpha = arctan(r*)
  gamma' = -pi/2*b2 + alpha*(2*b2-1),  phi = pi*neg - sgn*gamma'
With t clamped to ~0+, the lens formula reproduces all three branches of
the reference's `where` (separated -> ~0, contained -> pi*rmin^2).
Reciprocals go through Exp(-Ln(x)) (ACT Reciprocal/Rsqrt are banned);
penalty uses d/(d+s1) = sigmoid(0.5 ln d2 - ln s1).

ACT activations are emitted phase-grouped (sigmoid-set vs ln/exp-set)
to minimize ~2.7us activation-table reloads.  Scalar losses are
per-partition partial sums (free accum_out) finished on the host.
"""

import os as _os

import numpy as np

import concourse.bass as bass
import concourse.bacc as bacc
import concourse.mybir as mybir
from concourse.mybir import AluOpType as alu
from concourse.mybir import ActivationFunctionType as act
from concourse.tile import TileContext
from concourse.bass_utils import run_bass_kernel_spmd

F32 = mybir.dt.float32
BF = mybir.dt.bfloat16

# problem constants (hardcoded per contest rules)
B, A, G = 128, 3, 104
NCORES = 8
BC = B // NCORES            # 16 batches per core
GG = G * G                  # 10816
Q = 8                       # p = b*Q + q
FT = GG // Q                # 1352
NCH = 2
F = FT // NCH               # 676
PRA = [20.0 / 8.0, 50.0 / 8.0, 110.0 / 8.0]   # anchor radii in grid units
LNPRA = [float(np.log(v)) for v in PRA]
T01 = float(np.sqrt((20.0 * 50.0) / 64.0))    # riou argmax thresholds on tr*G
T12 = float(np.sqrt((50.0 * 110.0) / 64.0))
EPSG = 1e-9 * G * G           # reference EPS scaled to grid units^2
TINY = 1e-12
ALPHA = 0.25
PI = float(np.pi)

TRACE = False          # test.py can flip this
LAST_RESULTS = None    # test.py reads exec_time_ns from here

_CACHE = {}


def _register_const(nc, val):
    t = nc.alloc_sbuf_tensor(f"constf32-{val}", [128, 1], F32)
    nc.gpsimd.memset(t.ap(), val)
    nc.const_aps.aps[(F32, val)] = t.ap()


def _patch_act_tables():
    # Make Bacc's table-load inserter map Exp AND Ln to the combined
    # natural_log_exp_and_others set (ids must stay aligned with
    # act_info.json, so edit contents rather than order).
    if getattr(bacc, "_act_tables_patched", False):
        return
    orig = bacc.get_activation_tables

    def patched(module_arch):
        t = orig(module_arch)
        if "natural_log_exp_and_others" in t:
            exp = mybir.ActivationFunctionType.Exp
            ln = mybir.ActivationFunctionType.Ln
            for name, fns in t.items():
                if name == "natural_log_exp_and_others":
                    continue
                fns.discard(exp)
                fns.discard(ln)
        return t

    bacc.get_activation_tables = patched
    bacc._act_tables_patched = True


def _build_nc():
    _patch_act_tables()
    nc = bacc.Bacc("TRN2", target_bir_lowering=False, debug=False)
    for v in [EPSG, TINY] + LNPRA:
        _register_const(nc, v)
    nc.all_engine_barrier()

    x_d = nc.dram_tensor("x", [BC, 4 * A, Q, NCH, F], F32, kind="ExternalInput")
    tg_d = nc.dram_tensor("tg", [BC, 4, Q, NCH, F], F32, kind="ExternalInput")
    cst_d = nc.dram_tensor("cst", [2, NCH, 128, F], F32, kind="ExternalInput")
    out_d = nc.dram_tensor("out", [BC, A, Q, NCH, F, 4], F32, kind="ExternalOutput")
    part_d = nc.dram_tensor("part", [128, 16], F32, kind="ExternalOutput")

    with TileContext(nc) as tc:
        with (
            tc.tile_pool(name="const", bufs=1) as pconst,
            tc.tile_pool(name="tg", bufs=1) as ptg,
            tc.tile_pool(name="xin", bufs=2) as pxin,
            tc.tile_pool(name="head", bufs=1) as phead,
            tc.tile_pool(name="foc", bufs=1) as pfoc,
            tc.tile_pool(name="sel", bufs=1) as psel,
            tc.tile_pool(name="dlong", bufs=1) as pdl,
            tc.tile_pool(name="dshort", bufs=12) as pds,
            tc.tile_pool(name="dshortB", bufs=5) as pdsB,
            tc.tile_pool(name="outp", bufs=2) as pout,
            tc.tile_pool(name="accp", bufs=1) as pacc,
        ):
            accs = {}

            def acc_tile(j):
                t = pacc.tile([128, 1], F32, tag=f"acc{j}", name=f"acc{j}")
                accs[j] = t
                return t[:]

            for h in range(NCH):
                cx8 = pconst.tile([128, F], F32, tag="cx8", bufs=2, name=f"cx8_{h}")
                cy8 = pconst.tile([128, F], F32, tag="cy8", bufs=2, name=f"cy8_{h}")
                nc.sync.dma_start(cx8[:], cst_d.ap()[0, h])
                nc.sync.dma_start(cy8[:], cst_d.ap()[1, h])

                # ---- targets (DVE only) ----
                tgt = {}
                for i, nm in enumerate(["tx", "ty", "tr", "cf"]):
                    t = ptg.tile([128, F], F32, tag=nm, name=f"{nm}_{h}")
                    nc.sync.dma_start(t[:], tg_d.ap()[:, i, :, h, :])
                    tgt[nm] = t
                trg = tgt["tr"]
                nc.vector.tensor_scalar(trg[:], tgt["tr"][:], float(G), None, alu.mult)
                tr2g = ptg.tile([128, F], F32, tag="tr2g", name=f"tr2g_{h}")
                nc.vector.tensor_tensor(tr2g[:], trg[:], trg[:], alu.mult)
                m01 = ptg.tile([128, F], mybir.dt.uint8, tag="m01", name=f"m01_{h}")
                nc.vector.tensor_scalar(m01[:], trg[:], T01, None, alu.is_le)
                m12 = ptg.tile([128, F], mybir.dt.uint8, tag="m12", name=f"m12_{h}")
                nc.vector.tensor_scalar(m12[:], trg[:], T12, None, alu.is_le)
                nc.vector.tensor_reduce(acc_tile(h * 8 + 0), tgt["cf"][:],
                                        mybir.AxisListType.X, alu.add)
                cfb = ptg.tile([128, F], BF, tag="cfb", name=f"cfb_{h}")
                nc.gpsimd.tensor_copy(cfb[:], tgt["cf"][:])

                # ---- phase SIG-1: all head sigmoids ----
                ch = {}
                for a in range(A):
                    for i, nm in enumerate(["px", "py", "pr", "pc"]):
                        t = pxin.tile([128, F], F32, tag=nm,
                                      bufs=2 if nm == "pc" else 1,
                                      name=f"{nm}{a}_{h}")
                        nc.sync.dma_start(t[:], x_d.ap()[:, a * 4 + i, :, h, :])
                        ch[(a, nm)] = t
                sxs, sys_, brns, pts, Ls = [], [], [], [], []
                for a in range(A):
                    sx = phead.tile([128, F], F32, tag=f"sx{a}", name=f"sx{a}_{h}")
                    nc.scalar.activation(sx[:], ch[(a, "px")][:], act.Sigmoid)
                    sy = phead.tile([128, F], F32, tag=f"sy{a}", name=f"sy{a}_{h}")
                    nc.scalar.activation(sy[:], ch[(a, "py")][:], act.Sigmoid)
                    pt = pfoc.tile([128, F], F32, tag=f"pt{a}", name=f"pt{a}_{h}")
                    nc.scalar.activation(pt[:], ch[(a, "pc")][:], act.Sigmoid)
                    sxs.append(sx); sys_.append(sy); pts.append(pt)

                # ---- phase LNEXP-1: brn, L; head writes + dense focal (DVE) ----
                for a in range(A):
                    brn = phead.tile([128, F], F32, tag=f"brn{a}", name=f"brn{a}_{h}")
                    nc.scalar.activation(brn[:], ch[(a, "pr")][:], act.Exp, bias=LNPRA[a])
                    L = pfoc.tile([128, F], F32, tag=f"L{a}", name=f"L{a}_{h}")
                    nc.scalar.activation(L[:], pts[a][:], act.Ln)
                    brns.append(brn); Ls.append(L)

                corrs = []
                for a in range(A):
                    sx, sy, brn, pt, L = sxs[a], sys_[a], brns[a], pts[a], Ls[a]
                    ot = pout.tile([128, 4 * F], F32, tag="out", name=f"ot{a}_{h}")
                    nc.vector.scalar_tensor_tensor(
                        ot[:, 0:4 * F:4], sx[:], 8.0, cx8[:], alu.mult, alu.add)
                    nc.vector.scalar_tensor_tensor(
                        ot[:, 1:4 * F:4], sy[:], 8.0, cy8[:], alu.mult, alu.add)
                    nc.gpsimd.tensor_scalar(ot[:, 2:4 * F:4], brn[:], 8.0, None, alu.mult)
                    nc.gpsimd.tensor_copy(ot[:, 3:4 * F:4], pt[:])
                    ptb = pfoc.tile([128, F], BF, tag="ptb", name=f"ptb{a}_{h}")
                    nc.gpsimd.tensor_copy(ptb[:], pt[:])
                    nc.gpsimd.dma_start(out_d.ap()[:, a, :, h, :, :], ot[:])

                    # dense focal: t1 = pt^2 * (pc - L); corrS
                    w = pfoc.tile([128, F], F32, tag="w", bufs=2, name=f"w{a}_{h}")
                    nc.vector.scalar_tensor_tensor(
                        w[:], L[:], -1.0, ch[(a, "pc")][:], alu.mult, alu.add)
                    pt2 = pfoc.tile([128, F], F32, tag="ptsq", bufs=2, name=f"pt2{a}_{h}")
                    nc.vector.tensor_tensor(pt2[:], pt[:], pt[:], alu.mult)
                    ompt = pfoc.tile([128, F], F32, tag="ompt", bufs=2, name=f"ompt{a}_{h}")
                    nc.vector.tensor_scalar(ompt[:], pt[:], -1.0, None, alu.add)
                    omp2 = pfoc.tile([128, F], F32, tag="omp2", bufs=2, name=f"omp2{a}_{h}")
                    nc.vector.tensor_tensor(omp2[:], ompt[:], ompt[:], alu.mult)
                    u2n = pfoc.tile([128, F], F32, tag="u2n", bufs=2, name=f"u2n{a}_{h}")
                    nc.vector.tensor_tensor(u2n[:], omp2[:], L[:], alu.mult)
                    t1 = pfoc.tile([128, F], F32, tag="t1", bufs=2, name=f"t1{a}_{h}")
                    nc.vector.tensor_tensor(t1[:], pt2[:], w[:], alu.mult)
                    nc.vector.tensor_reduce(acc_tile(h * 8 + 2 + a), t1[:],
                                            mybir.AxisListType.X, alu.add)
                    cs = phead.tile([128, F], mybir.dt.bfloat16, tag=f"cs{a}", name=f"cs{a}_{h}")
                    nc.vector.scalar_tensor_tensor(
                        cs[:], u2n[:], -ALPHA / (1.0 - ALPHA), t1[:],
                        alu.mult, alu.subtract)
                    corrs.append(cs)

                # ---- best-anchor selection ----
                def select(srcs, tag):
                    vb = psel.tile([128, F], F32, tag=tag, name=f"{tag}_{h}")
                    nc.gpsimd.tensor_copy(vb[:], srcs[2][:])
                    nc.vector.copy_predicated(vb[:], m12[:], srcs[1][:])
                    nc.vector.copy_predicated(vb[:], m01[:], srcs[0][:])
                    return vb

                sxb = select(sxs, "sxb")
                syb = select(sys_, "syb")
                r1b = select(brns, "r1b")
                csb = psel.tile([128, F], mybir.dt.bfloat16, tag="csb", name=f"csb_{h}")
                nc.gpsimd.tensor_copy(csb[:], corrs[2][:])
                nc.vector.copy_predicated(csb[:], m12[:], corrs[1][:])
                nc.vector.copy_predicated(csb[:], m01[:], corrs[0][:])

                # ---- DIoU on best anchor (grid units), DVE part 1 ----
                ctr = [0]

                def dst(tag=None):
                    ctr[0] += 1
                    if tag is None:
                        return pds.tile([128, F], BF, tag="ds", name=f"ds{h}_{ctr[0]}")
                    if tag == "B":
                        return pdsB.tile([128, F], BF, tag="dsB", name=f"dsB{h}_{ctr[0]}")
                    return pdl.tile([128, F], BF, tag=tag, name=f"{tag}_{h}")

                dx = dst(); nc.vector.tensor_tensor(dx[:], sxb[:], txb[:], alu.subtract)
                dy = dst(); nc.vector.tensor_tensor(dy[:], syb[:], tyb[:], alu.subtract)
                dx2 = dst(); nc.vector.tensor_tensor(dx2[:], dx[:], dx[:], alu.mult)
                dy2 = dst(); nc.vector.tensor_tensor(dy2[:], dy[:], dy[:], alu.mult)
                d2 = dst("d2"); nc.vector.tensor_tensor(d2[:], dx2[:], dy2[:], alu.add)
                r1sq = dst("r1sq"); nc.vector.tensor_tensor(r1sq[:], r1b[:], r1b[:], alu.mult)
                rdiff = dst(); nc.vector.tensor_tensor(rdiff[:], r1sq[:], tr2g[:], alu.subtract)
                num1 = dst("num1"); nc.vector.tensor_tensor(num1[:], d2[:], rdiff[:], alu.add)
                num2 = dst("num2"); nc.vector.tensor_tensor(num2[:], d2[:], rdiff[:], alu.subtract)
                s1 = dst("s1"); nc.vector.tensor_tensor(s1[:], r1b[:], trg[:], alu.add)
                s2 = dst(); nc.vector.tensor_tensor(s2[:], r1b[:], trg[:], alu.subtract)
                s1sq = dst(); nc.vector.tensor_tensor(s1sq[:], s1[:], s1[:], alu.mult)
                s2sq = dst(); nc.vector.tensor_tensor(s2sq[:], s2[:], s2[:], alu.mult)
                uu = dst(); nc.vector.tensor_tensor(uu[:], s1sq[:], d2[:], alu.subtract)
                vv = dst(); nc.vector.tensor_tensor(vv[:], d2[:], s2sq[:], alu.subtract)
                tt = dst(); nc.vector.tensor_tensor(tt[:], uu[:], vv[:], alu.mult)
                tcl = dst(); nc.vector.tensor_scalar(tcl[:], tt[:], 0.0, None, alu.max)

                # ---- phase LNEXP-2 (Abs is in every table set) ----
                ua1 = dst(); nc.scalar.activation(ua1[:], num1[:], act.Abs)
                ua2 = dst("B"); nc.scalar.activation(ua2[:], num2[:], act.Abs)
                l_d2 = dst(); nc.scalar.activation(l_d2[:], d2[:], act.Ln, bias=EPSG)
                l_s1 = dst(); nc.scalar.activation(l_s1[:], s1[:], act.Ln)
                lnt = dst("lnt"); nc.scalar.activation(lnt[:], tcl[:], act.Ln, bias=TINY)
                sqt = dst("sqt"); nc.scalar.activation(sqt[:], lnt[:], act.Exp, scale=0.5)
                lu1 = dst(); nc.scalar.activation(lu1[:], ua1[:], act.Ln, bias=TINY)
                lu2 = dst("B"); nc.scalar.activation(lu2[:], ua2[:], act.Ln, bias=TINY)

                zp = dst()
                nc.vector.scalar_tensor_tensor(zp[:], l_d2[:], 0.5, l_s1[:], alu.mult, alu.subtract)
                dl1 = dst("dl1")
                nc.vector.scalar_tensor_tensor(dl1[:], lnt[:], -0.5, lu1[:], alu.mult, alu.add)
                dl2 = dst("dl2")
                nc.vector.scalar_tensor_tensor(dl2[:], lnt[:], -0.5, lu2[:], alu.mult, alu.add)

                adl1 = dst(); nc.scalar.activation(adl1[:], dl1[:], act.Abs)
                adl2 = dst("B"); nc.scalar.activation(adl2[:], dl2[:], act.Abs)
                rr1 = dst(); nc.scalar.activation(rr1[:], adl1[:], act.Exp, scale=-1.0)
                rr2 = dst("B"); nc.scalar.activation(rr2[:], adl2[:], act.Exp, scale=-1.0)

                # ---- phase SIG-2: arctans + penalty sigmoid ----
                # (al1/sg/al2 stay contiguous in the ACT stream: one table load)
                def phi_from(al, dl, num, tag, pool=None):
                    d_ = (lambda: dst("B")) if pool == "B" else (lambda: dst())
                    neg = d_(); nc.vector.tensor_scalar(neg[:], num[:], 0.0, None, alu.is_lt)
                    b2 = d_(); nc.vector.tensor_scalar(b2[:], dl[:], 0.0, None, alu.is_le)
                    b22 = d_(); nc.vector.tensor_scalar(b22[:], b2[:], 2.0, -1.0, alu.mult, alu.add)
                    sgn = d_(); nc.vector.tensor_scalar(sgn[:], neg[:], -2.0, 1.0, alu.mult, alu.add)
                    ab = d_(); nc.vector.tensor_tensor(ab[:], al[:], b22[:], alu.mult)
                    gp = d_()
                    nc.vector.scalar_tensor_tensor(gp[:], b2[:], -PI / 2.0, ab[:], alu.mult, alu.add)
                    sga = d_(); nc.vector.tensor_tensor(sga[:], sgn[:], gp[:], alu.mult)
                    phi = dst(tag)
                    nc.vector.scalar_tensor_tensor(phi[:], neg[:], PI, sga[:], alu.mult, alu.subtract)
                    return phi

                al1 = dst(); nc.scalar.activation(al1[:], rr1[:], act.Arctan)
                sg = dst("sg"); nc.scalar.activation(sg[:], zp[:], act.Sigmoid)
                ph1 = phi_from(al1, dl1, num1, "ph1")
                al2 = dst("B"); nc.scalar.activation(al2[:], rr2[:], act.Arctan)
                ph2 = phi_from(al2, dl2, num2, "ph2", pool="B")

                q1 = dst(); nc.vector.tensor_tensor(q1[:], r1sq[:], ph1[:], alu.mult)
                q2 = dst("B"); nc.vector.tensor_tensor(q2[:], tr2g[:], ph2[:], alu.mult)
                S_ = dst("S_"); nc.vector.tensor_tensor(S_[:], r1sq[:], tr2g[:], alu.add)
                c1 = dst(); nc.vector.tensor_tensor(c1[:], q1[:], q2[:], alu.add)
                inter = dst("inter")
                nc.vector.scalar_tensor_tensor(inter[:], sqt[:], -0.5, c1[:], alu.mult, alu.add)
                un = dst()
                nc.vector.scalar_tensor_tensor(un[:], S_[:], PI, inter[:], alu.mult, alu.subtract)

                # ---- phase LNEXP-3: 1/union ----
                l_un = dst(); nc.scalar.activation(l_un[:], un[:], act.Ln, bias=EPSG)
                iun = dst(); nc.scalar.activation(iun[:], l_un[:], act.Exp, scale=-1.0)

                # ---- DVE part 3: dio + masked accums ----
                iou = dst(); nc.vector.tensor_tensor(iou[:], inter[:], iun[:], alu.mult)
                pen = dst(); nc.vector.tensor_tensor(pen[:], sg[:], sg[:], alu.mult)
                dio = dst(); nc.vector.tensor_tensor(dio[:], pen[:], iou[:], alu.subtract)
                jk2 = dst()
                nc.vector.scalar_tensor_tensor(
                    jk2[:], dio[:], 1.0, cfb[:], alu.bypass, alu.mult,
                    accum_out=acc_tile(h * 8 + 1))
                jk3 = dst()
                nc.vector.scalar_tensor_tensor(
                    jk3[:], csb[:], 1.0, cfb[:], alu.bypass, alu.mult,
                    accum_out=acc_tile(h * 8 + 5))

            for j, t in accs.items():
                nc.sync.dma_start(part_d.ap()[:, j:j + 1], t[:])

    nc.compile()
    return nc


def _consts_np():
    # cst[0, h] = 8*gx, cst[1, h] = 8*gy for cell = q*1352 + h*676 + f,
    # at partition p = b*8 + q (independent of b).
    cst = np.zeros((2, NCH, 128, F), np.float32)
    p = np.arange(128)
    q = p % Q
    f = np.arange(F)
    for h in range(NCH):
        cell = q[:, None] * FT + h * F + f[None, :]
        cst[0, h] = 8.0 * (cell % G)
        cst[1, h] = 8.0 * (cell // G)
    return cst


def kernel(x, targets):
    global LAST_RESULTS
    if "nc" not in _CACHE:
        _CACHE["nc"] = _build_nc()
        _CACHE["cst"] = _consts_np()
    nc = _CACHE["nc"]
    cst = _CACHE["cst"]

    x = np.ascontiguousarray(np.asarray(x, np.float32))
    targets = np.ascontiguousarray(np.asarray(targets, np.float32))

    in_maps = []
    for c in range(NCORES):
        xs = x[c * BC:(c + 1) * BC].reshape(BC, 4 * A, Q, NCH, F)
        ts = targets[c * BC:(c + 1) * BC].reshape(BC, 4, Q, NCH, F)
        in_maps.append({"x": xs, "tg": ts, "cst": cst})

    res = run_bass_kernel_spmd(nc, in_maps, core_ids=list(range(NCORES)),
                               trace=TRACE)
    LAST_RESULTS = res

    out_full = np.concatenate(
        [r["out"].reshape(BC, A * GG, 4) for r in res.results], axis=0)

    parts = np.stack([r["part"] for r in res.results]).astype(np.float64)
    cols = parts.reshape(NCORES, 128, NCH, 8)
    nobj = cols[..., 0].sum()
    sc = cols[..., 1].sum()
    st1 = cols[..., 2:5].sum()
    smc = cols[..., 5].sum()
    loss_bbox = (sc + nobj) / max(nobj, 1.0)
    loss_conf = (1.0 - ALPHA) * (st1 + smc) / float(B * A * GG)
    return out_full, np.float32(loss_bbox + loss_conf)
